# revision 8
# baseline (speedup 1.0000x reference)
"""Trainium2 Bass kernel for ContentPopularityJointAttention.

Computes, for each batch row b:
    mp     = concat(m[b], p[b])            # (50, 512)
    hidden = tanh(mp @ Wu)                 # (50, 512)
    s      = hidden @ bvec                 # (50,)
    u[b]   = (sum_n s_n * m[b,n]) / (sum_n s_n)   # (256,)

Sharding: pure data parallel over the batch dim across 8 NeuronCores.

Precision notes (measured): the sum-normalized attention amplifies score
errors by ~1/|sum s|; the hidden matmul needs >=16 valid mantissa bits on
BOTH operands (fp16 1-term: 0.39 rel err; fp32r single-pass HW matmul has
~1.5e-4 product error -> ~0.2 rel err; both FAIL the 2e-2 gate). A 3-term
fp16 hi/lo split (xh@Wh + xl@Wh + xh@Wl) gives 6.8e-4. The pooling
NUMERATOR tolerates fp16 (2.9e-4) but the ones-column S (denominator)
must be true fp32.

Per-core dataflow (tokens = rows*50, 128-token chunks; PE is the
bottleneck at ~6276 cycles/chunk, all other engines hide under it):
  1. Host pre-splits x=concat(m,p) into fp16 hi/lo and pre-transposes to
     feature-major chunk-blocked layout mpT [128,C,4(dchunk),2(hi/lo),128]
     (one 2KB-per-partition-descriptor DMA per chunk; no PE transposes).
     Token-major m_hi [tok,256] f16 is DMA'd for the pooling stationary.
  2. 12 fp16 matmuls (3-term split, Wu moving, ap=512) -> hid PSUM f32.
  3. ACT tanh -> SBUF f32.
  4. DVE mul by b-replicated (fp32 products) + reduce -> s [128,1] f32.
  5. DVE s * block-diag row mask -> blk32 f32 and blk16 f16.
  6. PE pooling, flipped so the small mask side streams: two matmuls
     lhsT=mh half [128t,128d] (stationary), rhs=blk16 [128t,64r] fp16
     (ap=64 -> 64c each) -> uT PSUM [128d,64r] per d-half, plus
     lhsT=blk32 @ rhs=ones (fp32, ap=1 -> 4c) -> S PSUM [64,1],
     all accumulated over the 25 chunks of each 64-row group.
  7. Group end: ACT copies uT/S PSUM->SBUF, DMA out. The final
     u = uT.T / S normalization happens on the host during unshard
     (exact fp32 divide, zero device cost).
"""

import numpy as np
from contextlib import ExitStack

import concourse.bass as bass
import concourse.bacc as bacc
import concourse.tile as tile
from concourse import mybir
from concourse.bass_utils import run_bass_kernel_spmd

N_CORES = 8
B_FULL, N_TOK, MD, PD = 4096, 50, 256, 256
D = MD + PD          # 512 contraction dim
K = 512              # hidden dim
CHUNK = 128          # tokens per chunk (partition dim)
GROUP_ROWS = 64      # batch rows per pooling PSUM accumulation group
GROUP_CHUNKS = GROUP_ROWS * N_TOK // CHUNK   # 25
POOL_P = 64          # pooling free dim (rows per group; max local row 63)

f32 = mybir.dt.float32
f16 = mybir.dt.float16


def build_program(b_shard: int):
    """Build the single-core Bass program (SPMD: same program, all cores)."""
    tokens = b_shard * N_TOK
    assert tokens % (CHUNK * GROUP_CHUNKS) == 0
    n_groups = b_shard // GROUP_ROWS
    n_chunks = tokens // CHUNK

    nc = bacc.Bacc("TRN2", target_bir_lowering=False, debug=False,
                   num_devices=N_CORES)

    # feature-major fp16 hi/lo of concat(m,p), chunk-blocked:
    # mpT[q, c, j, h, t] = x_h[c*128+t, j*128+q]
    mpT_d = nc.dram_tensor("mpT", [128, n_chunks, 4, 2, CHUNK], f16,
                           kind="ExternalInput").ap()
    # token-major fp16(m) for the pooling stationary operand
    mhi_d = nc.dram_tensor("mhi", [tokens, MD], f16, kind="ExternalInput").ap()
    wu_hi_d = nc.dram_tensor("wu_hi", [128, 4, K], f16, kind="ExternalInput").ap()
    wu_lo_d = nc.dram_tensor("wu_lo", [128, 4, K], f16, kind="ExternalInput").ap()
    brep_d = nc.dram_tensor("brep", [128, K], f32, kind="ExternalInput").ap()
    masks_d = nc.dram_tensor("masks", [128, GROUP_CHUNKS, POOL_P], f32,
                             kind="ExternalInput").ap()
    masks16_d = nc.dram_tensor("masks16", [128, GROUP_CHUNKS, POOL_P], f16,
                               kind="ExternalInput").ap()
    ones_d = nc.dram_tensor("ones", [128, 1], f32, kind="ExternalInput").ap()
    # transposed pooled output + per-row score sums (host divides)
    uT_d = nc.dram_tensor("uT", [n_groups, 128, 2, POOL_P], f32,
                          kind="ExternalOutput").ap()
    sS_d = nc.dram_tensor("sS", [n_groups, POOL_P, 1], f32,
                          kind="ExternalOutput").ap()

    with tile.TileContext(nc) as tc, ExitStack() as ctx:
        singles = ctx.enter_context(tc.tile_pool(name="singles", bufs=1))
        io_x = ctx.enter_context(tc.tile_pool(name="iox", bufs=3))
        io_m = ctx.enter_context(tc.tile_pool(name="iom", bufs=3))
        io_u = ctx.enter_context(tc.tile_pool(name="iou", bufs=2))
        work = ctx.enter_context(tc.tile_pool(name="work", bufs=3))
        psum_h = ctx.enter_context(tc.tile_pool(name="psumH", bufs=2, space="PSUM"))
        psum_a = ctx.enter_context(tc.tile_pool(name="psumA", bufs=2, space="PSUM"))
        psum_b = ctx.enter_context(tc.tile_pool(name="psumB", bufs=2, space="PSUM"))
        psum_s = ctx.enter_context(tc.tile_pool(name="psumS", bufs=2, space="PSUM"))

        wu_hi_sb = singles.tile([128, 4, K], f16)
        nc.gpsimd.dma_start(out=wu_hi_sb[:], in_=wu_hi_d)
        wu_lo_sb = singles.tile([128, 4, K], f16)
        nc.gpsimd.dma_start(out=wu_lo_sb[:], in_=wu_lo_d)
        brep_sb = singles.tile([128, K], f32)
        nc.gpsimd.dma_start(out=brep_sb[:], in_=brep_d)
        masks_sb = singles.tile([128, GROUP_CHUNKS, POOL_P], f32)
        nc.gpsimd.dma_start(out=masks_sb[:], in_=masks_d)
        masks16_sb = singles.tile([128, GROUP_CHUNKS, POOL_P], f16)
        nc.gpsimd.dma_start(out=masks16_sb[:], in_=masks16_d)
        ones_sb = singles.tile([128, 1], f32)
        nc.gpsimd.dma_start(out=ones_sb[:], in_=ones_d)

        for g in range(n_groups):
            pool_a = psum_a.tile([128, POOL_P], f32)   # d 0..127
            pool_b = psum_b.tile([128, POOL_P], f32)   # d 128..255
            pool_s = psum_s.tile([POOL_P, 1], f32)
            for l in range(GROUP_CHUNKS):
                c = g * GROUP_CHUNKS + l
                t0 = c * CHUNK

                xT = io_x.tile([128, 4, 2, CHUNK], f16)
                nc.sync.dma_start(out=xT[:], in_=mpT_d[:, c])
                mh = io_m.tile([128, MD], f16)
                nc.scalar.dma_start(out=mh[:], in_=mhi_d[t0:t0 + CHUNK, :])

                # hidden = tanh(mp @ Wu), 3-term fp16 split
                hid = psum_h.tile([128, K], f32)
                i_mm = 0
                for h_x, wu_sb in ((0, wu_hi_sb), (1, wu_hi_sb), (0, wu_lo_sb)):
                    for j in range(4):
                        nc.tensor.matmul(
                            hid[:],
                            lhsT=xT[:, j, h_x, :],
                            rhs=wu_sb[:, j, :],
                            start=(i_mm == 0),
                            stop=(i_mm == 11),
                        )
                        i_mm += 1

                tanhH = work.tile([128, K], f32)
                nc.scalar.activation(out=tanhH[:], in_=hid[:],
                                     func=mybir.ActivationFunctionType.Tanh)

                # s[tok] = sum_k tanhH * b   (fp32 products on DVE)
                scr = work.tile([128, K], f32)
                s = work.tile([128, 1], f32)
                nc.vector.tensor_mul(scr[:], tanhH[:], brep_sb[:])
                nc.vector.reduce_sum(s[:], scr[:], axis=mybir.AxisListType.X)

                # block-diagonal pooling masks: fp32 for the S column
                # (cancellation-amplified), fp16 for the m pooling
                blk32 = work.tile([128, POOL_P], f32)
                nc.vector.tensor_scalar_mul(blk32[:], masks_sb[:, l, :], s[:])
                blk16 = work.tile([128, POOL_P], f16)
                nc.vector.tensor_scalar_mul(blk16[:], masks16_sb[:, l, :], s[:])

                nc.tensor.matmul(
                    pool_a[:],
                    lhsT=mh[:, 0:128],
                    rhs=blk16[:],
                    start=(l == 0),
                    stop=(l == GROUP_CHUNKS - 1),
                )
                nc.tensor.matmul(
                    pool_b[:],
                    lhsT=mh[:, 128:256],
                    rhs=blk16[:],
                    start=(l == 0),
                    stop=(l == GROUP_CHUNKS - 1),
                )
                nc.tensor.matmul(
                    pool_s[:],
                    lhsT=blk32[:],
                    rhs=ones_sb[:],
                    start=(l == 0),
                    stop=(l == GROUP_CHUNKS - 1),
                )

            u_sb = io_u.tile([128, 2, POOL_P], f32)
            nc.scalar.copy(out=u_sb[:, 0, :], in_=pool_a[:])
            nc.scalar.copy(out=u_sb[:, 1, :], in_=pool_b[:])
            s_sb = io_u.tile([POOL_P, 1], f32)
            nc.vector.tensor_copy(out=s_sb[:], in_=pool_s[:])
            nc.sync.dma_start(out=uT_d[g], in_=u_sb[:])
            nc.sync.dma_start(out=sS_d[g], in_=s_sb[:])

    nc.compile()
    return nc


def host_constants(Wu: np.ndarray, b: np.ndarray):
    Wu = np.asarray(Wu, np.float32)
    b = np.asarray(b, np.float32)
    wu_hi16 = Wu.astype(np.float16)
    wu_lo16 = (Wu - wu_hi16.astype(np.float32)).astype(np.float16)
    # [d, k] -> [d%128, d//128, k]
    wu_hi = np.ascontiguousarray(wu_hi16.reshape(4, 128, K).transpose(1, 0, 2))
    wu_lo = np.ascontiguousarray(wu_lo16.reshape(4, 128, K).transpose(1, 0, 2))
    brep = np.ascontiguousarray(np.broadcast_to(b, (128, K)))
    tp = np.arange(128)[:, None, None]
    ll = np.arange(GROUP_CHUNKS)[None, :, None]
    rr = np.arange(POOL_P)[None, None, :]
    masks = (((CHUNK * ll + tp) // N_TOK) == rr).astype(np.float32)
    ones = np.ones((128, 1), np.float32)
    return {"wu_hi": wu_hi, "wu_lo": wu_lo, "brep": brep, "masks": masks,
            "masks16": masks.astype(np.float16), "ones": ones}


def host_shard_inputs(m_shard: np.ndarray, p_shard: np.ndarray):
    """Per-shard data tensors: fp16 hi/lo feature-major chunk-blocked mpT
    and token-major fp16 m for the pooling stationary operand."""
    tokens = m_shard.shape[0] * N_TOK
    n_chunks = tokens // CHUNK
    x = np.concatenate(
        [m_shard.reshape(tokens, MD), p_shard.reshape(tokens, PD)], axis=1)
    xh = x.astype(np.float16)
    xl = (x - xh.astype(np.float32)).astype(np.float16)
    # [tok, 512] -> [128q, n_chunks, 4j, 128t]
    def to_fmajor(a):
        return a.reshape(n_chunks, CHUNK, 4, 128).transpose(3, 0, 2, 1)
    mpT = np.ascontiguousarray(
        np.stack([to_fmajor(xh), to_fmajor(xl)], axis=3))
    mhi = np.ascontiguousarray(xh[:, 0:MD])
    return {"mpT": mpT, "mhi": mhi}


def unshard_output(uT: np.ndarray, sS: np.ndarray) -> np.ndarray:
    """[n_groups,128,2,64] pooled sums + [n_groups,64,1] score sums ->
    normalized u [rows, 256]."""
    n_groups = uT.shape[0]
    # uT[g, q, h, r] -> u[g*64+r, h*128+q]
    u = uT.transpose(0, 3, 2, 1).reshape(n_groups * POOL_P, MD)
    S = sS.reshape(n_groups * POOL_P, 1)
    return u / S


_prog_cache: dict = {}


def get_program(b_shard: int):
    if b_shard not in _prog_cache:
        _prog_cache[b_shard] = build_program(b_shard)
    return _prog_cache[b_shard]


def kernel(m: np.ndarray, p: np.ndarray, Wu: np.ndarray, b: np.ndarray
           ) -> np.ndarray:
    m = np.ascontiguousarray(np.asarray(m, np.float32))
    p = np.ascontiguousarray(np.asarray(p, np.float32))
    B = m.shape[0]
    assert B % N_CORES == 0
    b_shard = B // N_CORES

    nc = get_program(b_shard)
    consts = host_constants(Wu, b)

    in_maps = []
    for c in range(N_CORES):
        ms = m[c * b_shard:(c + 1) * b_shard]
        ps = p[c * b_shard:(c + 1) * b_shard]
        in_maps.append({**host_shard_inputs(ms, ps), **consts})
    res = run_bass_kernel_spmd(nc, in_maps, list(range(N_CORES)))
    u = np.concatenate(
        [unshard_output(res.results[c]["uT"], res.results[c]["sS"])
         for c in range(N_CORES)], axis=0)
    return u.astype(np.float32)


# revision 11
# speedup vs baseline: 1.0641x; 1.0641x over previous
"""Trainium2 Bass kernel for ContentPopularityJointAttention.

Computes, for each batch row b:
    mp     = concat(m[b], p[b])            # (50, 512)
    hidden = tanh(mp @ Wu)                 # (50, 512)
    s      = hidden @ bvec                 # (50,)
    u[b]   = (sum_n s_n * m[b,n]) / (sum_n s_n)   # (256,)

Sharding: pure data parallel over the batch dim across 8 NeuronCores.

Precision notes (measured): the sum-normalized attention amplifies score
errors by ~1/|sum s|; the hidden matmul needs >=16 valid mantissa bits on
BOTH operands (fp16 1-term: 0.39 rel err; fp32r single-pass HW matmul has
~1.5e-4 product error -> ~0.2 rel err; both FAIL the 2e-2 gate). A 3-term
fp16 hi/lo split (xh@Wh + xl@Wh + xh@Wl) gives 6.8e-4. The pooling
NUMERATOR tolerates fp16 (2.9e-4) but the ones-column S (denominator)
must be true fp32.

Per-core dataflow (tokens = rows*50, 128-token chunks; PE is the
bottleneck at ~6276 cycles/chunk, all other engines hide under it):
  1. Host pre-splits x=concat(m,p) into fp16 hi/lo and pre-transposes to
     feature-major chunk-blocked layout mpT [128,C,4(dchunk),2(hi/lo),128]
     (one 2KB-per-partition-descriptor DMA per chunk; no PE transposes).
     Token-major m_hi [tok,256] f16 is DMA'd for the pooling stationary.
  2. 12 fp16 matmuls (3-term split, Wu moving, ap=512) -> hid PSUM f32.
  3. ACT tanh -> SBUF f32.
  4. DVE mul by b-replicated (fp32 products) + reduce -> s [128,1] f32.
  5. DVE s * block-diag row mask -> blk32 f32 and blk16 f16.
  6. PE pooling, flipped so the small mask side streams: two matmuls
     lhsT=mh half [128t,128d] (stationary), rhs=blk16 [128t,64r] fp16
     (ap=64 -> 64c each) -> uT PSUM [128d,64r] per d-half, plus
     lhsT=blk32 @ rhs=ones (fp32, ap=1 -> 4c) -> S PSUM [64,1],
     all accumulated over the 25 chunks of each 64-row group.
  7. Group end: ACT copies uT/S PSUM->SBUF, DMA out. The final
     u = uT.T / S normalization happens on the host during unshard
     (exact fp32 divide, zero device cost).
"""

import numpy as np
from contextlib import ExitStack

import concourse.bass as bass
import concourse.bacc as bacc
import concourse.tile as tile
from concourse import mybir
from concourse.bass_utils import run_bass_kernel_spmd

N_CORES = 8
B_FULL, N_TOK, MD, PD = 4096, 50, 256, 256
D = MD + PD          # 512 contraction dim
K = 512              # hidden dim
CHUNK = 128          # tokens per chunk (partition dim)
GROUP_ROWS = 64      # batch rows per pooling PSUM accumulation group
GROUP_CHUNKS = GROUP_ROWS * N_TOK // CHUNK   # 25
POOL_P = 64          # pooling free dim (rows per group; max local row 63)
# Hidden-matmul correction terms (xl@Wh, xh@Wl) cover only the KEEP
# hidden columns with the largest |b_k| (host sorts Wu/b columns by b_k;
# the score sum over k is permutation-invariant). Dropping the bottom 48
# columns' corrections costs 5.6e-3 rel err (measured; gate 2e-2) and
# saves 2*4*(512-KEEP) PE cycles per chunk.
KEEP = 464

f32 = mybir.dt.float32
f16 = mybir.dt.float16


def build_program(b_shard: int):
    """Build the single-core Bass program (SPMD: same program, all cores)."""
    tokens = b_shard * N_TOK
    assert tokens % (CHUNK * GROUP_CHUNKS) == 0
    n_groups = b_shard // GROUP_ROWS
    n_chunks = tokens // CHUNK

    nc = bacc.Bacc("TRN2", target_bir_lowering=False, debug=False,
                   num_devices=N_CORES)

    # feature-major fp16 hi/lo of concat(m,p), chunk-blocked:
    # mpT[q, c, j, h, t] = x_h[c*128+t, j*128+q]
    mpT_d = nc.dram_tensor("mpT", [128, n_chunks, 4, 2, CHUNK], f16,
                           kind="ExternalInput").ap()
    # token-major fp16(m) for the pooling stationary operand
    mhi_d = nc.dram_tensor("mhi", [tokens, MD], f16, kind="ExternalInput").ap()
    wu_hi_d = nc.dram_tensor("wu_hi", [128, 4, K], f16, kind="ExternalInput").ap()
    wu_lo_d = nc.dram_tensor("wu_lo", [128, 4, K], f16, kind="ExternalInput").ap()
    brep_d = nc.dram_tensor("brep", [128, K], f32, kind="ExternalInput").ap()
    masks_d = nc.dram_tensor("masks", [128, GROUP_CHUNKS, POOL_P], f32,
                             kind="ExternalInput").ap()
    masks16_d = nc.dram_tensor("masks16", [128, GROUP_CHUNKS, POOL_P], f16,
                               kind="ExternalInput").ap()
    ones_d = nc.dram_tensor("ones", [128, 1], f32, kind="ExternalInput").ap()
    # transposed pooled output + per-row score sums (host divides)
    uT_d = nc.dram_tensor("uT", [n_groups, 128, 2, POOL_P], f32,
                          kind="ExternalOutput").ap()
    sS_d = nc.dram_tensor("sS", [n_groups, POOL_P, 1], f32,
                          kind="ExternalOutput").ap()

    with tile.TileContext(nc) as tc, ExitStack() as ctx:
        singles = ctx.enter_context(tc.tile_pool(name="singles", bufs=1))
        io_x = ctx.enter_context(tc.tile_pool(name="iox", bufs=3))
        io_m = ctx.enter_context(tc.tile_pool(name="iom", bufs=3))
        io_u = ctx.enter_context(tc.tile_pool(name="iou", bufs=2))
        work = ctx.enter_context(tc.tile_pool(name="work", bufs=3))
        psum_h = ctx.enter_context(tc.tile_pool(name="psumH", bufs=2, space="PSUM"))
        psum_a = ctx.enter_context(tc.tile_pool(name="psumA", bufs=2, space="PSUM"))
        psum_b = ctx.enter_context(tc.tile_pool(name="psumB", bufs=2, space="PSUM"))
        psum_s = ctx.enter_context(tc.tile_pool(name="psumS", bufs=2, space="PSUM"))

        wu_hi_sb = singles.tile([128, 4, K], f16)
        nc.gpsimd.dma_start(out=wu_hi_sb[:], in_=wu_hi_d)
        wu_lo_sb = singles.tile([128, 4, K], f16)
        nc.gpsimd.dma_start(out=wu_lo_sb[:], in_=wu_lo_d)
        brep_sb = singles.tile([128, K], f32)
        nc.gpsimd.dma_start(out=brep_sb[:], in_=brep_d)
        masks_sb = singles.tile([128, GROUP_CHUNKS, POOL_P], f32)
        nc.gpsimd.dma_start(out=masks_sb[:], in_=masks_d)
        masks16_sb = singles.tile([128, GROUP_CHUNKS, POOL_P], f16)
        nc.gpsimd.dma_start(out=masks16_sb[:], in_=masks16_d)
        ones_sb = singles.tile([128, 1], f32)
        nc.gpsimd.dma_start(out=ones_sb[:], in_=ones_d)

        for g in range(n_groups):
            pool_a = psum_a.tile([128, POOL_P], f32)   # d 0..127
            pool_b = psum_b.tile([128, POOL_P], f32)   # d 128..255
            pool_s = psum_s.tile([POOL_P, 1], f32)
            for l in range(GROUP_CHUNKS):
                c = g * GROUP_CHUNKS + l
                t0 = c * CHUNK

                xT = io_x.tile([128, 4, 2, CHUNK], f16)
                nc.sync.dma_start(out=xT[:], in_=mpT_d[:, c])
                mh = io_m.tile([128, MD], f16)
                nc.scalar.dma_start(out=mh[:], in_=mhi_d[t0:t0 + CHUNK, :])

                # hidden = tanh(mp @ Wu), 3-term fp16 split
                hid = psum_h.tile([128, K], f32)
                i_mm = 0
                for h_x, wu_sb, kw in ((0, wu_hi_sb, K), (1, wu_hi_sb, KEEP),
                                       (0, wu_lo_sb, KEEP)):
                    for j in range(4):
                        nc.tensor.matmul(
                            hid[:, 0:kw],
                            lhsT=xT[:, j, h_x, :],
                            rhs=wu_sb[:, j, 0:kw],
                            start=(i_mm == 0),
                            stop=(i_mm == 11),
                        )
                        i_mm += 1

                tanhH = work.tile([128, K], f32)
                nc.scalar.activation(out=tanhH[:], in_=hid[:],
                                     func=mybir.ActivationFunctionType.Tanh)

                # s[tok] = sum_k tanhH * b   (fp32 products on DVE)
                scr = work.tile([128, K], f32)
                s = work.tile([128, 1], f32)
                nc.vector.tensor_mul(scr[:], tanhH[:], brep_sb[:])
                nc.vector.reduce_sum(s[:], scr[:], axis=mybir.AxisListType.X)

                # block-diagonal pooling masks: fp32 for the S column
                # (cancellation-amplified), fp16 for the m pooling
                blk32 = work.tile([128, POOL_P], f32)
                nc.vector.tensor_scalar_mul(blk32[:], masks_sb[:, l, :], s[:])
                blk16 = work.tile([128, POOL_P], f16)
                nc.vector.tensor_scalar_mul(blk16[:], masks16_sb[:, l, :], s[:])

                nc.tensor.matmul(
                    pool_a[:],
                    lhsT=mh[:, 0:128],
                    rhs=blk16[:],
                    start=(l == 0),
                    stop=(l == GROUP_CHUNKS - 1),
                )
                nc.tensor.matmul(
                    pool_b[:],
                    lhsT=mh[:, 128:256],
                    rhs=blk16[:],
                    start=(l == 0),
                    stop=(l == GROUP_CHUNKS - 1),
                )
                nc.tensor.matmul(
                    pool_s[:],
                    lhsT=blk32[:],
                    rhs=ones_sb[:],
                    start=(l == 0),
                    stop=(l == GROUP_CHUNKS - 1),
                )

            u_sb = io_u.tile([128, 2, POOL_P], f32)
            nc.scalar.copy(out=u_sb[:, 0, :], in_=pool_a[:])
            nc.scalar.copy(out=u_sb[:, 1, :], in_=pool_b[:])
            s_sb = io_u.tile([POOL_P, 1], f32)
            nc.vector.tensor_copy(out=s_sb[:], in_=pool_s[:])
            nc.sync.dma_start(out=uT_d[g], in_=u_sb[:])
            nc.sync.dma_start(out=sS_d[g], in_=s_sb[:])

    nc.compile()
    return nc


def host_constants(Wu: np.ndarray, b: np.ndarray):
    Wu = np.asarray(Wu, np.float32)
    b = np.asarray(b, np.float32)
    # sort hidden columns by |b_k| descending so truncated correction
    # matmuls (first KEEP columns) cover the highest-weight columns
    order = np.argsort(-np.abs(b))
    Wu = np.ascontiguousarray(Wu[:, order])
    b = np.ascontiguousarray(b[order])
    wu_hi16 = Wu.astype(np.float16)
    wu_lo16 = (Wu - wu_hi16.astype(np.float32)).astype(np.float16)
    # [d, k] -> [d%128, d//128, k]
    wu_hi = np.ascontiguousarray(wu_hi16.reshape(4, 128, K).transpose(1, 0, 2))
    wu_lo = np.ascontiguousarray(wu_lo16.reshape(4, 128, K).transpose(1, 0, 2))
    brep = np.ascontiguousarray(np.broadcast_to(b, (128, K)))
    tp = np.arange(128)[:, None, None]
    ll = np.arange(GROUP_CHUNKS)[None, :, None]
    rr = np.arange(POOL_P)[None, None, :]
    masks = (((CHUNK * ll + tp) // N_TOK) == rr).astype(np.float32)
    ones = np.ones((128, 1), np.float32)
    return {"wu_hi": wu_hi, "wu_lo": wu_lo, "brep": brep, "masks": masks,
            "masks16": masks.astype(np.float16), "ones": ones}


def host_shard_inputs(m_shard: np.ndarray, p_shard: np.ndarray):
    """Per-shard data tensors: fp16 hi/lo feature-major chunk-blocked mpT
    and token-major fp16 m for the pooling stationary operand."""
    tokens = m_shard.shape[0] * N_TOK
    n_chunks = tokens // CHUNK
    x = np.concatenate(
        [m_shard.reshape(tokens, MD), p_shard.reshape(tokens, PD)], axis=1)
    xh = x.astype(np.float16)
    xl = (x - xh.astype(np.float32)).astype(np.float16)
    # [tok, 512] -> [128q, n_chunks, 4j, 128t]
    def to_fmajor(a):
        return a.reshape(n_chunks, CHUNK, 4, 128).transpose(3, 0, 2, 1)
    mpT = np.ascontiguousarray(
        np.stack([to_fmajor(xh), to_fmajor(xl)], axis=3))
    mhi = np.ascontiguousarray(xh[:, 0:MD])
    return {"mpT": mpT, "mhi": mhi}


def unshard_output(uT: np.ndarray, sS: np.ndarray) -> np.ndarray:
    """[n_groups,128,2,64] pooled sums + [n_groups,64,1] score sums ->
    normalized u [rows, 256]."""
    n_groups = uT.shape[0]
    # uT[g, q, h, r] -> u[g*64+r, h*128+q]
    u = uT.transpose(0, 3, 2, 1).reshape(n_groups * POOL_P, MD)
    S = sS.reshape(n_groups * POOL_P, 1)
    return u / S


_prog_cache: dict = {}


def get_program(b_shard: int):
    if b_shard not in _prog_cache:
        _prog_cache[b_shard] = build_program(b_shard)
    return _prog_cache[b_shard]


def kernel(m: np.ndarray, p: np.ndarray, Wu: np.ndarray, b: np.ndarray
           ) -> np.ndarray:
    m = np.ascontiguousarray(np.asarray(m, np.float32))
    p = np.ascontiguousarray(np.asarray(p, np.float32))
    B = m.shape[0]
    assert B % N_CORES == 0
    b_shard = B // N_CORES

    nc = get_program(b_shard)
    consts = host_constants(Wu, b)

    in_maps = []
    for c in range(N_CORES):
        ms = m[c * b_shard:(c + 1) * b_shard]
        ps = p[c * b_shard:(c + 1) * b_shard]
        in_maps.append({**host_shard_inputs(ms, ps), **consts})
    res = run_bass_kernel_spmd(nc, in_maps, list(range(N_CORES)))
    u = np.concatenate(
        [unshard_output(res.results[c]["uT"], res.results[c]["sS"])
         for c in range(N_CORES)], axis=0)
    return u.astype(np.float32)


# revision 12
# speedup vs baseline: 1.0850x; 1.0196x over previous
"""Trainium2 Bass kernel for ContentPopularityJointAttention.

Computes, for each batch row b:
    mp     = concat(m[b], p[b])            # (50, 512)
    hidden = tanh(mp @ Wu)                 # (50, 512)
    s      = hidden @ bvec                 # (50,)
    u[b]   = (sum_n s_n * m[b,n]) / (sum_n s_n)   # (256,)

Sharding: pure data parallel over the batch dim across 8 NeuronCores.

Precision notes (measured): the sum-normalized attention amplifies score
errors by ~1/|sum s|; the hidden matmul needs >=16 valid mantissa bits on
BOTH operands (fp16 1-term: 0.39 rel err; fp32r single-pass HW matmul has
~1.5e-4 product error -> ~0.2 rel err; both FAIL the 2e-2 gate). A 3-term
fp16 hi/lo split (xh@Wh + xl@Wh + xh@Wl) gives 6.8e-4. The pooling
NUMERATOR tolerates fp16 (2.9e-4) but the ones-column S (denominator)
must be true fp32.

Per-core dataflow (tokens = rows*50, 128-token chunks; PE is the
bottleneck at ~6276 cycles/chunk, all other engines hide under it):
  1. Host pre-splits x=concat(m,p) into fp16 hi/lo and pre-transposes to
     feature-major chunk-blocked layout mpT [128,C,4(dchunk),2(hi/lo),128]
     (one 2KB-per-partition-descriptor DMA per chunk; no PE transposes).
     Token-major m_hi [tok,256] f16 is DMA'd for the pooling stationary.
  2. 12 fp16 matmuls (3-term split, Wu moving, ap=512) -> hid PSUM f32.
  3. ACT tanh -> SBUF f32.
  4. DVE mul by b-replicated (fp32 products) + reduce -> s [128,1] f32.
  5. DVE s * block-diag row mask -> blk32 f32 and blk16 f16.
  6. PE pooling, flipped so the small mask side streams: two matmuls
     lhsT=mh half [128t,128d] (stationary), rhs=blk16 [128t,64r] fp16
     (ap=64 -> 64c each) -> uT PSUM [128d,64r] per d-half, plus
     lhsT=blk32 @ rhs=ones (fp32, ap=1 -> 4c) -> S PSUM [64,1],
     all accumulated over the 25 chunks of each 64-row group.
  7. Group end: ACT copies uT/S PSUM->SBUF, DMA out. The final
     u = uT.T / S normalization happens on the host during unshard
     (exact fp32 divide, zero device cost).
"""

import numpy as np
from contextlib import ExitStack

import concourse.bass as bass
import concourse.bacc as bacc
import concourse.tile as tile
from concourse import mybir
from concourse.bass_utils import run_bass_kernel_spmd

N_CORES = 8
B_FULL, N_TOK, MD, PD = 4096, 50, 256, 256
D = MD + PD          # 512 contraction dim
K = 512              # hidden dim
CHUNK = 128          # tokens per chunk (partition dim)
GROUP_ROWS = 64      # batch rows per pooling PSUM accumulation group
GROUP_CHUNKS = GROUP_ROWS * N_TOK // CHUNK   # 25
POOL_P = 64          # pooling free dim (rows per group; max local row 63)
# Hidden-matmul correction terms (xl@Wh, xh@Wl) cover only the KEEP
# hidden columns with the largest |b_k| (host sorts Wu/b columns by b_k;
# the score sum over k is permutation-invariant). Dropping the bottom 64
# columns' corrections costs 9.4e-3 rel err (measured; gate 2e-2) and
# saves 2*4*(512-KEEP) PE cycles per chunk.
KEEP = 448

f32 = mybir.dt.float32
f16 = mybir.dt.float16


def build_program(b_shard: int):
    """Build the single-core Bass program (SPMD: same program, all cores)."""
    tokens = b_shard * N_TOK
    assert tokens % (CHUNK * GROUP_CHUNKS) == 0
    n_groups = b_shard // GROUP_ROWS
    n_chunks = tokens // CHUNK

    nc = bacc.Bacc("TRN2", target_bir_lowering=False, debug=False,
                   num_devices=N_CORES)

    # feature-major fp16 hi/lo of concat(m,p), chunk-blocked:
    # mpT[q, c, j, h, t] = x_h[c*128+t, j*128+q]
    mpT_d = nc.dram_tensor("mpT", [128, n_chunks, 4, 2, CHUNK], f16,
                           kind="ExternalInput").ap()
    # token-major fp16(m) for the pooling stationary operand
    mhi_d = nc.dram_tensor("mhi", [tokens, MD], f16, kind="ExternalInput").ap()
    wu_hi_d = nc.dram_tensor("wu_hi", [128, 4, K], f16, kind="ExternalInput").ap()
    wu_lo_d = nc.dram_tensor("wu_lo", [128, 4, K], f16, kind="ExternalInput").ap()
    brep_d = nc.dram_tensor("brep", [128, K], f32, kind="ExternalInput").ap()
    masks_d = nc.dram_tensor("masks", [128, GROUP_CHUNKS, POOL_P], f32,
                             kind="ExternalInput").ap()
    masks16_d = nc.dram_tensor("masks16", [128, GROUP_CHUNKS, POOL_P], f16,
                               kind="ExternalInput").ap()
    ones_d = nc.dram_tensor("ones", [128, 1], f32, kind="ExternalInput").ap()
    # transposed pooled output + per-row score sums (host divides)
    uT_d = nc.dram_tensor("uT", [n_groups, 128, 2, POOL_P], f32,
                          kind="ExternalOutput").ap()
    sS_d = nc.dram_tensor("sS", [n_groups, POOL_P, 1], f32,
                          kind="ExternalOutput").ap()

    with tile.TileContext(nc) as tc, ExitStack() as ctx:
        singles = ctx.enter_context(tc.tile_pool(name="singles", bufs=1))
        io_x = ctx.enter_context(tc.tile_pool(name="iox", bufs=3))
        io_m = ctx.enter_context(tc.tile_pool(name="iom", bufs=3))
        io_u = ctx.enter_context(tc.tile_pool(name="iou", bufs=2))
        work = ctx.enter_context(tc.tile_pool(name="work", bufs=3))
        psum_h = ctx.enter_context(tc.tile_pool(name="psumH", bufs=2, space="PSUM"))
        psum_a = ctx.enter_context(tc.tile_pool(name="psumA", bufs=2, space="PSUM"))
        psum_b = ctx.enter_context(tc.tile_pool(name="psumB", bufs=2, space="PSUM"))
        psum_s = ctx.enter_context(tc.tile_pool(name="psumS", bufs=2, space="PSUM"))

        wu_hi_sb = singles.tile([128, 4, K], f16)
        nc.gpsimd.dma_start(out=wu_hi_sb[:], in_=wu_hi_d)
        wu_lo_sb = singles.tile([128, 4, K], f16)
        nc.gpsimd.dma_start(out=wu_lo_sb[:], in_=wu_lo_d)
        brep_sb = singles.tile([128, K], f32)
        nc.gpsimd.dma_start(out=brep_sb[:], in_=brep_d)
        masks_sb = singles.tile([128, GROUP_CHUNKS, POOL_P], f32)
        nc.gpsimd.dma_start(out=masks_sb[:], in_=masks_d)
        masks16_sb = singles.tile([128, GROUP_CHUNKS, POOL_P], f16)
        nc.gpsimd.dma_start(out=masks16_sb[:], in_=masks16_d)
        ones_sb = singles.tile([128, 1], f32)
        nc.gpsimd.dma_start(out=ones_sb[:], in_=ones_d)

        for g in range(n_groups):
            pool_a = psum_a.tile([128, POOL_P], f32)   # d 0..127
            pool_b = psum_b.tile([128, POOL_P], f32)   # d 128..255
            pool_s = psum_s.tile([POOL_P, 1], f32)
            for l in range(GROUP_CHUNKS):
                c = g * GROUP_CHUNKS + l
                t0 = c * CHUNK

                xT = io_x.tile([128, 4, 2, CHUNK], f16)
                nc.sync.dma_start(out=xT[:], in_=mpT_d[:, c])
                mh = io_m.tile([128, MD], f16)
                nc.scalar.dma_start(out=mh[:], in_=mhi_d[t0:t0 + CHUNK, :])

                # hidden = tanh(mp @ Wu), 3-term fp16 split
                hid = psum_h.tile([128, K], f32)
                i_mm = 0
                for h_x, wu_sb, kw in ((0, wu_hi_sb, K), (1, wu_hi_sb, KEEP),
                                       (0, wu_lo_sb, KEEP)):
                    for j in range(4):
                        nc.tensor.matmul(
                            hid[:, 0:kw],
                            lhsT=xT[:, j, h_x, :],
                            rhs=wu_sb[:, j, 0:kw],
                            start=(i_mm == 0),
                            stop=(i_mm == 11),
                        )
                        i_mm += 1

                tanhH = work.tile([128, K], f32)
                nc.scalar.activation(out=tanhH[:], in_=hid[:],
                                     func=mybir.ActivationFunctionType.Tanh)

                # s[tok] = sum_k tanhH * b   (fp32 products on DVE)
                scr = work.tile([128, K], f32)
                s = work.tile([128, 1], f32)
                nc.vector.tensor_mul(scr[:], tanhH[:], brep_sb[:])
                nc.vector.reduce_sum(s[:], scr[:], axis=mybir.AxisListType.X)

                # block-diagonal pooling masks: fp32 for the S column
                # (cancellation-amplified), fp16 for the m pooling
                blk32 = work.tile([128, POOL_P], f32)
                nc.vector.tensor_scalar_mul(blk32[:], masks_sb[:, l, :], s[:])
                blk16 = work.tile([128, POOL_P], f16)
                nc.vector.tensor_scalar_mul(blk16[:], masks16_sb[:, l, :], s[:])

                nc.tensor.matmul(
                    pool_a[:],
                    lhsT=mh[:, 0:128],
                    rhs=blk16[:],
                    start=(l == 0),
                    stop=(l == GROUP_CHUNKS - 1),
                )
                nc.tensor.matmul(
                    pool_b[:],
                    lhsT=mh[:, 128:256],
                    rhs=blk16[:],
                    start=(l == 0),
                    stop=(l == GROUP_CHUNKS - 1),
                )
                nc.tensor.matmul(
                    pool_s[:],
                    lhsT=blk32[:],
                    rhs=ones_sb[:],
                    start=(l == 0),
                    stop=(l == GROUP_CHUNKS - 1),
                )

            u_sb = io_u.tile([128, 2, POOL_P], f32)
            nc.scalar.copy(out=u_sb[:, 0, :], in_=pool_a[:])
            nc.scalar.copy(out=u_sb[:, 1, :], in_=pool_b[:])
            s_sb = io_u.tile([POOL_P, 1], f32)
            nc.vector.tensor_copy(out=s_sb[:], in_=pool_s[:])
            nc.sync.dma_start(out=uT_d[g], in_=u_sb[:])
            nc.sync.dma_start(out=sS_d[g], in_=s_sb[:])

    nc.compile()
    return nc


def host_constants(Wu: np.ndarray, b: np.ndarray):
    Wu = np.asarray(Wu, np.float32)
    b = np.asarray(b, np.float32)
    # sort hidden columns by |b_k| descending so truncated correction
    # matmuls (first KEEP columns) cover the highest-weight columns
    order = np.argsort(-np.abs(b))
    Wu = np.ascontiguousarray(Wu[:, order])
    b = np.ascontiguousarray(b[order])
    wu_hi16 = Wu.astype(np.float16)
    wu_lo16 = (Wu - wu_hi16.astype(np.float32)).astype(np.float16)
    # [d, k] -> [d%128, d//128, k]
    wu_hi = np.ascontiguousarray(wu_hi16.reshape(4, 128, K).transpose(1, 0, 2))
    wu_lo = np.ascontiguousarray(wu_lo16.reshape(4, 128, K).transpose(1, 0, 2))
    brep = np.ascontiguousarray(np.broadcast_to(b, (128, K)))
    tp = np.arange(128)[:, None, None]
    ll = np.arange(GROUP_CHUNKS)[None, :, None]
    rr = np.arange(POOL_P)[None, None, :]
    masks = (((CHUNK * ll + tp) // N_TOK) == rr).astype(np.float32)
    ones = np.ones((128, 1), np.float32)
    return {"wu_hi": wu_hi, "wu_lo": wu_lo, "brep": brep, "masks": masks,
            "masks16": masks.astype(np.float16), "ones": ones}


def host_shard_inputs(m_shard: np.ndarray, p_shard: np.ndarray):
    """Per-shard data tensors: fp16 hi/lo feature-major chunk-blocked mpT
    and token-major fp16 m for the pooling stationary operand."""
    tokens = m_shard.shape[0] * N_TOK
    n_chunks = tokens // CHUNK
    x = np.concatenate(
        [m_shard.reshape(tokens, MD), p_shard.reshape(tokens, PD)], axis=1)
    xh = x.astype(np.float16)
    xl = (x - xh.astype(np.float32)).astype(np.float16)
    # [tok, 512] -> [128q, n_chunks, 4j, 128t]
    def to_fmajor(a):
        return a.reshape(n_chunks, CHUNK, 4, 128).transpose(3, 0, 2, 1)
    mpT = np.ascontiguousarray(
        np.stack([to_fmajor(xh), to_fmajor(xl)], axis=3))
    mhi = np.ascontiguousarray(xh[:, 0:MD])
    return {"mpT": mpT, "mhi": mhi}


def unshard_output(uT: np.ndarray, sS: np.ndarray) -> np.ndarray:
    """[n_groups,128,2,64] pooled sums + [n_groups,64,1] score sums ->
    normalized u [rows, 256]."""
    n_groups = uT.shape[0]
    # uT[g, q, h, r] -> u[g*64+r, h*128+q]
    u = uT.transpose(0, 3, 2, 1).reshape(n_groups * POOL_P, MD)
    S = sS.reshape(n_groups * POOL_P, 1)
    return u / S


_prog_cache: dict = {}


def get_program(b_shard: int):
    if b_shard not in _prog_cache:
        _prog_cache[b_shard] = build_program(b_shard)
    return _prog_cache[b_shard]


def kernel(m: np.ndarray, p: np.ndarray, Wu: np.ndarray, b: np.ndarray
           ) -> np.ndarray:
    m = np.ascontiguousarray(np.asarray(m, np.float32))
    p = np.ascontiguousarray(np.asarray(p, np.float32))
    B = m.shape[0]
    assert B % N_CORES == 0
    b_shard = B // N_CORES

    nc = get_program(b_shard)
    consts = host_constants(Wu, b)

    in_maps = []
    for c in range(N_CORES):
        ms = m[c * b_shard:(c + 1) * b_shard]
        ps = p[c * b_shard:(c + 1) * b_shard]
        in_maps.append({**host_shard_inputs(ms, ps), **consts})
    res = run_bass_kernel_spmd(nc, in_maps, list(range(N_CORES)))
    u = np.concatenate(
        [unshard_output(res.results[c]["uT"], res.results[c]["sS"])
         for c in range(N_CORES)], axis=0)
    return u.astype(np.float32)
